# revision 1
# baseline (speedup 1.0000x reference)
"""Distributed multi-head attention kernel for Trainium2 (8 NeuronCores).

Reference computation (EMBED=1024, HEADS=16, b=2, n=2048):
    qkv = x @ w_qkv.T                       -> [b, n, h, d, 3] (qkv innermost)
    q, k, v per head; energy = q @ k^T
    att = softmax(energy, -1) / sqrt(1024)
    out = att @ v -> [b, n, 1024]
    relu(out @ w_proj.T + b_proj)

Sharding: 2-way data parallel over batch x 4-way tensor parallel over heads.
Core c handles batch c//4, heads [4*(c%4) .. 4*(c%4)+3].  After attention,
each 4-core batch group AllGathers the per-core attention output features
(2 MB each) and every core computes a 256-feature slice of the output
projection (the slice is chosen purely by the host-side weight sharding, so
the SPMD program is rank-independent).

Layouts (all transposed so no on-device transposes are needed):
    xT        [1024, 2048]      x[batch].T           (matmul rhs / lhsT)
    qT,kT     [64, 2048]/head   via w_qT as lhsT, xT as rhs
    energy^T  [k, q] tiles      via kT as lhsT, qT as rhs
    softmax   exp on ACT; denominators via an extra ones-column in V
              (PV matmul computes [out^T ; sum_k exp] in one accumulation)
    out^T     [64, 2048]/head   normalized by DMA-broadcast reciprocal
    proj^T    [256, 2048]       w_projT slice as lhsT, gathered out^T as rhs

All matmuls run in float32r (TF32-like, ~1e-4 rel err, full PE rate).
The softmax-then-scale quirk (divide by sqrt(e) AFTER softmax) is folded
into w_proj on the host (w_proj / 32).
"""

import os
import sys
import types

sys.path.insert(0, "/opt/trn_rl_repo")

import numpy as np


def _install_ntff_shim():
    """The agent image's antenv lacks axon_hooks; recreate it so
    run_bass_kernel_spmd(trace=True) can capture NTFF profiles."""
    try:
        import antenv.axon_hooks  # noqa: F401
        return
    except ImportError:
        pass
    try:
        import antenv
        from trn_agent_boot.trn_boot import _ntff_profile_via_ctypes
    except ImportError:
        return
    mod = types.ModuleType("antenv.axon_hooks")
    _hook = [None]
    mod.set_axon_ntff_profile_hook = lambda h: _hook.__setitem__(0, h)
    mod.get_axon_ntff_profile_hook = lambda: _hook[0]
    sys.modules["antenv.axon_hooks"] = mod
    antenv.axon_hooks = mod
    mod.set_axon_ntff_profile_hook(
        _ntff_profile_via_ctypes("/opt/axon/libaxon_pjrt.so")
    )


_install_ntff_shim()

import concourse.bacc as bacc
import concourse.bass as bass
import concourse.tile as tile
from concourse import mybir
from concourse.bass_utils import run_bass_kernel_spmd

B, N, E, H, D = 2, 2048, 1024, 16, 64
NCORES = 8
GROUPS = [[0, 1, 2, 3], [4, 5, 6, 7]]
HPC = H // 4            # heads per core = 4
FC = HPC * D            # attention-output features per core = 256
QKV_F = 3 * FC          # qkv features per core = 768
ET = E // 128           # 8 k-tiles over the embed dim
NT = N // 512           # 4 n-tiles of 512
KT = N // 128           # 16 k-tiles of 128 over sequence
F32 = mybir.dt.float32
F32R = mybir.dt.float32r

LAST_EXEC_NS = None
LAST_RESULTS = None

_CACHED_NC = None


def _build():
    nc = bacc.Bacc("TRN2", target_bir_lowering=False, num_devices=NCORES)

    # Inputs (per-core shards, host-prepared).  float32r declarations let the
    # fast HWDGE sync DMA feed the PE directly (verified bit-identical to the
    # gpsimd cast path).
    xt_d = nc.dram_tensor("xt", [ET, 128, N], F32R, kind="ExternalInput")
    wqkv_d = nc.dram_tensor("wqkvt", [ET, 128, QKV_F], F32R, kind="ExternalInput")
    wproj_d = nc.dram_tensor("wprojt", [ET, 128, FC], F32R, kind="ExternalInput")
    bias_d = nc.dram_tensor("bias", [FC], F32, kind="ExternalInput")
    out_d = nc.dram_tensor("out", [FC, N], F32, kind="ExternalOutput")

    with tile.TileContext(nc) as tc:
        with (
            tc.tile_pool(name="persist", bufs=1) as persist,
            tc.tile_pool(name="dram", bufs=1, space="DRAM") as dram,
        ):
            # ---- persistent SBUF tensors -------------------------------
            wqkv_sb = persist.tile([128, ET, QKV_F], F32R)
            for kt in range(ET):
                nc.sync.dma_start(out=wqkv_sb[:, kt, :], in_=wqkv_d[kt])
            # wproj/bias loads are issued AFTER phase 1 below so they queue
            # behind the latency-critical x/w_qkv DMAs.
            wproj_sb = persist.tile([128, ET, FC], F32R)
            bias_sb = persist.tile([128, 2], F32)

            # q/k features of head pair p (2 heads x 64) on partitions;
            # [128, pair, n]
            qt_sb = persist.tile([128, 2, N], F32R)
            kt_sb = persist.tile([128, 2, N], F32R)
            # v in [n, d] layout + a ones column per head: slot = [64 v | 1]
            v_sb = persist.tile([128, KT, HPC, 65], F32R)
            ones_col = nc.const_aps.tensor(1.0, [128, KT, HPC, 1], F32)
            nc.vector.tensor_copy(v_sb[:, :, :, 64:65], ones_col)

            # per-quarter-n-chunk DRAM bounce buffers for the AllGather
            # (8 chunks of 256 columns for finer comm/compute overlap)
            ot_ch = [dram.tile([FC, 256], F32R, name=f"ot{i}") for i in range(2 * NT)]
            og_ch = [
                dram.tile([4 * FC, 256], F32R, name=f"og{i}") for i in range(2 * NT)
            ]

            # ---- phase 1: QKV projections ------------------------------
            with (
                tc.tile_pool(name="xtp", bufs=2) as xtp,
                tc.tile_pool(name="qkps", bufs=2, space="PSUM") as qkps_pool,
                tc.tile_pool(name="vps", bufs=2, space="PSUM") as vps_pool,
            ):
                for nt in range(NT):
                    xt_t = xtp.tile([128, ET, 512], F32R, tag="xt")
                    for kt in range(ET):
                        nc.sync.dma_start(
                            out=xt_t[:, kt, :],
                            in_=xt_d[kt, :, nt * 512 : (nt + 1) * 512],
                        )
                    for pair in range(2):
                        qps = qkps_pool.tile([128, 512], F32, tag="qk")
                        for kt in range(ET):
                            nc.tensor.matmul(
                                qps[:],
                                lhsT=wqkv_sb[:, kt, pair * 128 : (pair + 1) * 128],
                                rhs=xt_t[:, kt, :],
                                start=(kt == 0),
                                stop=(kt == ET - 1),
                            )
                        nc.vector.tensor_copy(
                            qt_sb[:, pair, nt * 512 : (nt + 1) * 512], qps[:]
                        )
                        kps = qkps_pool.tile([128, 512], F32, tag="qk")
                        for kt in range(ET):
                            nc.tensor.matmul(
                                kps[:],
                                lhsT=wqkv_sb[
                                    :, kt, 256 + pair * 128 : 256 + (pair + 1) * 128
                                ],
                                rhs=xt_t[:, kt, :],
                                start=(kt == 0),
                                stop=(kt == ET - 1),
                            )
                        nc.vector.tensor_copy(
                            kt_sb[:, pair, nt * 512 : (nt + 1) * 512], kps[:]
                        )
                    for m in range(4):
                        ns = nt * 4 + m
                        vps = vps_pool.tile([128, FC], F32, tag="v")
                        for kt in range(ET):
                            nc.tensor.matmul(
                                vps[:],
                                lhsT=xt_t[:, kt, m * 128 : (m + 1) * 128],
                                rhs=wqkv_sb[:, kt, 512:768],
                                start=(kt == 0),
                                stop=(kt == ET - 1),
                            )
                        nc.vector.tensor_copy(
                            v_sb[:, ns, :, 0:64],
                            vps[:].rearrange("p (h d) -> p h d", h=HPC),
                        )

            # weights for the projection tail can load behind everything else
            for kt in range(ET):
                nc.sync.dma_start(out=wproj_sb[:, kt, :], in_=wproj_d[kt])
            nc.sync.dma_start(
                out=bias_sb, in_=bias_d[:].rearrange("(g p) -> p g", p=128)
            )

            # ---- phase 2: attention + AllGather + projection -----------
            # Heads are processed in pairs: head a lives on partitions 0-63,
            # head b on 64-127 of the q/k tiles, so the two K=64 energy
            # matmuls occupy disjoint PE row-groups and run concurrently.
            with (
                tc.tile_pool(name="eps", bufs=2, space="PSUM") as eps_pool,
                tc.tile_pool(name="pvps", bufs=3, space="PSUM") as pvps_pool,
                tc.tile_pool(name="expp", bufs=3) as expp,
                tc.tile_pool(name="normp", bufs=4) as normp,
                tc.tile_pool(name="prjps", bufs=1, space="PSUM") as prjps_pool,
                tc.tile_pool(name="prhs", bufs=2) as prhs_pool,
                tc.tile_pool(name="outp", bufs=3) as outp,
            ):
                def emit_proj(ch):
                    # projection for one 256-column chunk (after its AllGather)
                    rhs_t = prhs_pool.tile([128, ET, 256], F32R, tag="prhs")
                    for kt in range(ET):
                        nc.sync.dma_start(
                            out=rhs_t[:, kt, :],
                            in_=og_ch[ch][kt * 128 : (kt + 1) * 128, :],
                        )
                    n_sl = slice(ch * 256, (ch + 1) * 256)
                    for mg in range(2):
                        pps = prjps_pool.tile([128, 256], F32, tag="pp")
                        for kt in range(ET):
                            nc.tensor.matmul(
                                pps[:],
                                lhsT=wproj_sb[:, kt, mg * 128 : (mg + 1) * 128],
                                rhs=rhs_t[:, kt, :],
                                start=(kt == 0),
                                stop=(kt == ET - 1),
                            )
                        ob = outp.tile([128, 256], F32, tag="ob")
                        nc.vector.tensor_scalar(
                            ob[:],
                            pps[:],
                            bias_sb[:, mg : mg + 1],
                            0.0,
                            mybir.AluOpType.add,
                            mybir.AluOpType.max,
                        )
                        nc.sync.dma_start(
                            out=out_d[mg * 128 : (mg + 1) * 128, n_sl],
                            in_=ob[:],
                        )

                for qt in range(NT):
                    q_sl = slice(qt * 512, (qt + 1) * 512)
                    for pair in range(2):
                        heads = (2 * pair, 2 * pair + 1)
                        pvs = [
                            pvps_pool.tile([128, 512], F32, tag="pv", name=f"pv{s}")
                            for s in range(2)
                        ]
                        for kt in range(KT):
                            eps = eps_pool.tile([128, 1024], F32, tag="e")
                            for s in range(2):
                                d_sl = slice(s * 64, (s + 1) * 64)
                                nc.tensor.matmul(
                                    eps[:, s * 512 : (s + 1) * 512],
                                    lhsT=kt_sb[d_sl, pair, kt * 128 : (kt + 1) * 128],
                                    rhs=qt_sb[d_sl, pair, q_sl],
                                    start=True,
                                    stop=True,
                                )
                            exp_t = expp.tile([128, 1024], F32R, tag="exp")
                            nc.scalar.activation(
                                exp_t[:], eps[:], mybir.ActivationFunctionType.Exp
                            )
                            for s in range(2):
                                nc.tensor.matmul(
                                    pvs[s][0:65, :],
                                    lhsT=v_sb[:, kt, heads[s], :],
                                    rhs=exp_t[:, s * 512 : (s + 1) * 512],
                                    start=(kt == 0),
                                    stop=(kt == KT - 1),
                                )
                        for s in range(2):
                            h, pv = heads[s], pvs[s]
                            # normalize out^T[d, q] by 1/denom[q]: bounce the
                            # denominator row through DRAM, broadcast-read it
                            # across 64 partitions, reciprocal + multiply wide.
                            den_sb = normp.tile([1, 512], F32, tag="den_sb")
                            nc.vector.tensor_copy(den_sb[:], pv[64:65, :])
                            den_d = dram.tile([1, 512], F32, tag="den", bufs=4)
                            nc.sync.dma_start(out=den_d[:], in_=den_sb[:])
                            rep = normp.tile([64, 512], F32, tag="rep")
                            nc.sync.dma_start(
                                out=rep[:],
                                in_=bass.AP(
                                    tensor=den_d.tensor,
                                    offset=den_d.offset,
                                    ap=[[0, 64], [1, 512]],
                                ),
                            )
                            reprec = normp.tile([64, 512], F32, tag="reprec")
                            nc.vector.reciprocal(reprec[:], rep[:])
                            o_sb = normp.tile([64, 512], F32R, tag="o")
                            nc.vector.tensor_mul(o_sb[:], pv[0:64, :], reprec[:])
                            for hf in range(2):
                                nc.sync.dma_start(
                                    out=ot_ch[2 * qt + hf][h * 64 : (h + 1) * 64, :],
                                    in_=o_sb[:, hf * 256 : (hf + 1) * 256],
                                )
                    # fire this qt's AllGathers now; emit its projection work
                    # one qt LATER so the FIFO engine queues never idle behind
                    # an in-flight collective.
                    for hf in range(2):
                        ch = 2 * qt + hf
                        nc.gpsimd.collective_compute(
                            "AllGather",
                            mybir.AluOpType.bypass,
                            replica_groups=GROUPS,
                            ins=[ot_ch[ch].opt()],
                            outs=[og_ch[ch].opt()],
                        )
                    if qt >= 1:
                        emit_proj(2 * (qt - 1))
                        emit_proj(2 * (qt - 1) + 1)
                emit_proj(2 * (NT - 1))
                emit_proj(2 * (NT - 1) + 1)

    nc.compile()
    return nc


def _get_nc():
    global _CACHED_NC
    if _CACHED_NC is None:
        _CACHED_NC = _build()
    return _CACHED_NC


def _prep_inputs(x, w_qkv, w_proj, b_proj):
    """Shard + relayout the full inputs for the 8 cores."""
    x = np.asarray(x, dtype=np.float32)
    w_qkv = np.asarray(w_qkv, dtype=np.float32)
    w_proj = np.asarray(w_proj, dtype=np.float32)
    b_proj = np.asarray(b_proj, dtype=np.float32)

    # x^T per batch: [E, N] -> tiles [ET, 128, N]
    xts = [
        np.ascontiguousarray(x[b].T).reshape(ET, 128, N) for b in range(B)
    ]
    # w_qkv rows are (h, d, qkv)-interleaved with qkv innermost
    wr = w_qkv.reshape(H, D, 3, E)
    # fold the post-softmax 1/sqrt(E) scaling into w_proj
    wp = w_proj / np.sqrt(E).astype(np.float32)

    wqkv_shards, wproj_shards, bias_shards = [], [], []
    for r in range(4):
        heads = range(4 * r, 4 * r + 4)
        qrows = np.concatenate([wr[h, :, 0, :] for h in heads], 0)  # [256, E]
        krows = np.concatenate([wr[h, :, 1, :] for h in heads], 0)
        vrows = np.concatenate([wr[h, :, 2, :] for h in heads], 0)
        w_core = np.concatenate([qrows, krows, vrows], 0)  # [768, E]
        wqkv_shards.append(
            np.ascontiguousarray(w_core.T).reshape(ET, 128, QKV_F)
        )
        wproj_shards.append(
            np.ascontiguousarray(wp[r * FC : (r + 1) * FC, :].T).reshape(
                ET, 128, FC
            )
        )
        bias_shards.append(np.ascontiguousarray(b_proj[r * FC : (r + 1) * FC]))

    in_maps = []
    for c in range(NCORES):
        b, r = c // 4, c % 4
        in_maps.append(
            {
                "xt": xts[b],
                "wqkvt": wqkv_shards[r],
                "wprojt": wproj_shards[r],
                "bias": bias_shards[r],
            }
        )
    return in_maps


def kernel(x, w_qkv, w_proj, b_proj):
    global LAST_EXEC_NS, LAST_RESULTS
    nc = _get_nc()
    in_maps = _prep_inputs(x, w_qkv, w_proj, b_proj)
    trace = bool(int(os.environ.get("BASS_KERNEL_TRACE", "0")))
    res = run_bass_kernel_spmd(
        nc, in_maps, list(range(NCORES)), trace=trace
    )
    LAST_EXEC_NS = res.exec_time_ns
    LAST_RESULTS = res

    out = np.empty((B, N, E), dtype=np.float32)
    for g in range(B):
        pt = np.concatenate(
            [res.results[4 * g + r]["out"] for r in range(4)], axis=0
        )  # [1024 f, 2048 n]
        out[g] = pt.T
    return out



# revision 3
# speedup vs baseline: 1.2774x; 1.2774x over previous
"""Distributed multi-head attention kernel for Trainium2 (8 NeuronCores).

Reference computation (EMBED=1024, HEADS=16, b=2, n=2048):
    qkv = x @ w_qkv.T                       -> [b, n, h, d, 3] (qkv innermost)
    q, k, v per head; energy = q @ k^T
    att = softmax(energy, -1) / sqrt(1024)
    out = att @ v -> [b, n, 1024]
    relu(out @ w_proj.T + b_proj)

Sharding: 2-way data parallel over batch x 4-way tensor parallel over heads.
Core c handles batch c//4, heads [4*(c%4) .. 4*(c%4)+3].  After attention,
each 4-core batch group AllGathers the per-core attention output features
and every core computes a 256-feature slice of the output projection.

v2 design (vs the fp32r v1 at 462us):
  * fp16 for the q/k path (x, w_qkv, q, k): 1 cycle/col on the PE vs the
    ~1.5-3x passes fp32/fp32r matmuls cost; rel err ~3e-3 (validated on host).
  * bf16 for everything softmax-onward (exp, v, att out, w_proj, AllGather,
    final output): exp needs bf16 range (values up to e^+45).
  * Single fused pipeline: K/V production interleaves with qt=0's attention
    (kt follows nt availability), Q[qt] and proj[qt-1] are emitted inside
    later qt's kt loop as PE gap fillers.  The ACT engine (exp, ~143us of
    [128,1024] activations) is the target critical path; the PE stream is
    kept dense so HAM stays warm.
  * PSUM budget (8 banks): energy pool [128,1024]f32 x2 (4 banks, shared
    with Q/K/V/proj chunk borrows), PV accumulators [65,512]f32 x4 (4 banks,
    live across each qt's kt loop; row 64 = ones-column softmax denominator).
  * Softmax normalization: denominators scatter-DMA'd to [128,8] so the DVE
    reciprocal runs 128-wide (v1 ran it 64x redundant on a broadcast tile:
    53us of DVE), then broadcast back over 64 partitions for one multiply.
  * AllGather in bf16 (halves wire bytes), one per qt (4 total).
"""

import os
import sys
import types

sys.path.insert(0, "/opt/trn_rl_repo")

import numpy as np
import ml_dtypes


def _install_ntff_shim():
    """The agent image's antenv lacks axon_hooks; recreate it so
    run_bass_kernel_spmd(trace=True) can capture NTFF profiles."""
    try:
        import antenv.axon_hooks  # noqa: F401
        return
    except ImportError:
        pass
    try:
        import antenv
        from trn_agent_boot.trn_boot import _ntff_profile_via_ctypes
    except ImportError:
        return
    mod = types.ModuleType("antenv.axon_hooks")
    _hook = [None]
    mod.set_axon_ntff_profile_hook = lambda h: _hook.__setitem__(0, h)
    mod.get_axon_ntff_profile_hook = lambda: _hook[0]
    sys.modules["antenv.axon_hooks"] = mod
    antenv.axon_hooks = mod
    mod.set_axon_ntff_profile_hook(
        _ntff_profile_via_ctypes("/opt/axon/libaxon_pjrt.so")
    )


_install_ntff_shim()

import concourse.bacc as bacc
import concourse.bass as bass
import concourse.tile as tile
from concourse import mybir
from concourse.bass_utils import run_bass_kernel_spmd

B, N, E, H, D = 2, 2048, 1024, 16, 64
NCORES = 8
GROUPS = [[0, 1, 2, 3], [4, 5, 6, 7]]
HPC = H // 4            # heads per core = 4
FC = HPC * D            # attention-output features per core = 256
QKV_F = 3 * FC          # qkv features per core = 768
ET = E // 128           # 8 k-tiles over the embed dim
NT = N // 512           # 4 n-tiles of 512
KT = N // 128           # 16 k-tiles of 128 over sequence
F32 = mybir.dt.float32
F16 = mybir.dt.float16
BF16 = mybir.dt.bfloat16

LAST_EXEC_NS = None
LAST_RESULTS = None

_CACHED_NC = None


def _build():
    nc = bacc.Bacc("TRN2", target_bir_lowering=False, num_devices=NCORES)

    xt_d = nc.dram_tensor("xt", [ET, 128, N], F16, kind="ExternalInput")
    wqkv_d = nc.dram_tensor("wqkvt", [ET, 128, QKV_F], F16, kind="ExternalInput")
    wproj_d = nc.dram_tensor("wprojt", [ET, 128, FC], BF16, kind="ExternalInput")
    bias_d = nc.dram_tensor("bias", [FC], F32, kind="ExternalInput")
    out_d = nc.dram_tensor("out", [FC, N], BF16, kind="ExternalOutput")

    with tile.TileContext(nc) as tc:
        with (
            tc.tile_pool(name="persist", bufs=1) as persist,
            tc.tile_pool(name="dram", bufs=1, space="DRAM") as dram,
            tc.tile_pool(name="xtp", bufs=NT) as xtp,
            tc.tile_pool(name="eps", bufs=2, space="PSUM") as eps_pool,
            tc.tile_pool(name="pvps", bufs=4, space="PSUM") as pvps_pool,
            tc.tile_pool(name="expp", bufs=4) as expp,
            tc.tile_pool(name="normp", bufs=2) as normp,
            tc.tile_pool(name="prhs", bufs=2) as prhs_pool,
            tc.tile_pool(name="outp", bufs=2) as outp,
        ):
            # ---- persistent SBUF tensors -------------------------------
            wqkv_sb = persist.tile([128, ET, QKV_F], F16)
            for kt in range(ET):
                nc.sync.dma_start(out=wqkv_sb[:, kt, :], in_=wqkv_d[kt])
            wproj_sb = persist.tile([128, ET, FC], BF16)
            bias_sb = persist.tile([128, 2], F32)

            # q/k features of head pair p (2 heads x 64d) on partitions
            qt_sb = persist.tile([128, 2, N], F16)
            kt_sb = persist.tile([128, 2, N], F16)
            # v in [n, d] layout + a ones column per head: slot = [64 v | 1]
            v_sb = persist.tile([128, KT, HPC, 65], BF16)
            ones_col = nc.const_aps.tensor(1.0, [128, KT, HPC, 1], F32)
            nc.vector.tensor_copy(v_sb[:, :, :, 64:65], ones_col)

            # DRAM bounce buffers
            ot_ch = [dram.tile([FC, 512], BF16, name=f"ot{i}") for i in range(NT)]
            og_ch = [
                dram.tile([4 * FC, 512], BF16, name=f"og{i}") for i in range(NT)
            ]

            xts = []

            # ---- emitters ---------------------------------------------
            def emit_x_load(nt):
                xt_t = xtp.tile([128, ET, 512], F16, tag="xt")
                xts.append(xt_t)
                for kt in range(ET):
                    nc.sync.dma_start(
                        out=xt_t[:, kt, :],
                        in_=xt_d[kt, :, nt * 512 : (nt + 1) * 512],
                    )

            def emit_qk(nt, pair, which):
                # which: 0 -> q, 1 -> k
                ps = eps_pool.tile([128, 512], F32, tag="e")
                off = which * 256 + pair * 128
                for kt in range(ET):
                    nc.tensor.matmul(
                        ps[:],
                        lhsT=wqkv_sb[:, kt, off : off + 128],
                        rhs=xts[nt][:, kt, :],
                        start=(kt == 0),
                        stop=(kt == ET - 1),
                    )
                dst = qt_sb if which == 0 else kt_sb
                nc.vector.tensor_copy(
                    dst[:, pair, nt * 512 : (nt + 1) * 512], ps[:]
                )

            def emit_v(nt, m):
                ps = eps_pool.tile([128, FC], F32, tag="e")
                for kt in range(ET):
                    nc.tensor.matmul(
                        ps[:],
                        lhsT=xts[nt][:, kt, m * 128 : (m + 1) * 128],
                        rhs=wqkv_sb[:, kt, 512:768],
                        start=(kt == 0),
                        stop=(kt == ET - 1),
                    )
                nc.vector.tensor_copy(
                    v_sb[:, nt * 4 + m, :, 0:64],
                    ps[:].rearrange("p (h d) -> p h d", h=HPC),
                )

            def attn_slot(qt, kt, pvt):
                # pvt: list of 4 per-head PV psum accumulators [65, 512]
                q_sl = slice(qt * 512, (qt + 1) * 512)
                exps = []
                for pair in range(2):
                    ep = eps_pool.tile([128, 1024], F32, tag="e")
                    for s in range(2):
                        d_sl = slice(s * 64, (s + 1) * 64)
                        nc.tensor.matmul(
                            ep[:, s * 512 : (s + 1) * 512],
                            lhsT=kt_sb[d_sl, pair, kt * 128 : (kt + 1) * 128],
                            rhs=qt_sb[d_sl, pair, q_sl],
                            start=True,
                            stop=True,
                        )
                    ex = expp.tile([128, 1024], BF16, tag="exp")
                    nc.scalar.activation(
                        ex[:], ep[:], mybir.ActivationFunctionType.Exp
                    )
                    exps.append(ex)
                for pair in range(2):
                    for s in range(2):
                        nc.tensor.matmul(
                            pvt[2 * pair + s][0:65, :],
                            lhsT=v_sb[:, kt, 2 * pair + s, :],
                            rhs=exps[pair][:, s * 512 : (s + 1) * 512],
                            start=(kt == 0),
                            stop=(kt == KT - 1),
                        )

            def emit_norm(qt, pvt):
                # normalize out^T[d, q] by 1/denominator[q] and store to the
                # AllGather input chunk.  reciprocal runs on a [128, 8]
                # scatter so the DVE is full-width.
                for pair in range(2):
                    den_sb = normp.tile([1, 1024], F32, tag="den_sb")
                    for s in range(2):
                        nc.vector.tensor_copy(
                            den_sb[:, s * 512 : (s + 1) * 512],
                            pvt[2 * pair + s][64:65, :],
                        )
                    den_d = dram.tile([1, 1024], F32, tag="den", bufs=4)
                    nc.sync.dma_start(out=den_d[:], in_=den_sb[:])
                    den_sc = normp.tile([128, 8], F32, tag="den_sc")
                    nc.sync.dma_start(
                        out=den_sc[:],
                        in_=bass.AP(
                            tensor=den_d.tensor,
                            offset=den_d.offset,
                            ap=[[8, 128], [1, 8]],
                        ),
                    )
                    rec_sc = normp.tile([128, 8], F32, tag="rec_sc")
                    nc.vector.reciprocal(rec_sc[:], den_sc[:])
                    rec_d = dram.tile([1, 1024], F32, tag="rec", bufs=4)
                    nc.sync.dma_start(
                        out=bass.AP(
                            tensor=rec_d.tensor,
                            offset=rec_d.offset,
                            ap=[[8, 128], [1, 8]],
                        ),
                        in_=rec_sc[:],
                    )
                    rep = normp.tile([64, 1024], F32, tag="rep")
                    nc.sync.dma_start(
                        out=rep[:],
                        in_=bass.AP(
                            tensor=rec_d.tensor,
                            offset=rec_d.offset,
                            ap=[[0, 64], [1, 1024]],
                        ),
                    )
                    o_sb = normp.tile([64, 1024], BF16, tag="o")
                    for s in range(2):
                        c_sl = slice(s * 512, (s + 1) * 512)
                        nc.vector.tensor_mul(
                            o_sb[:, c_sl], pvt[2 * pair + s][0:64, :], rep[:, c_sl]
                        )
                        h = 2 * pair + s
                        nc.sync.dma_start(
                            out=ot_ch[qt][h * 64 : (h + 1) * 64, :],
                            in_=o_sb[:, c_sl],
                        )

            def emit_ag(qt):
                nc.gpsimd.collective_compute(
                    "AllGather",
                    mybir.AluOpType.bypass,
                    replica_groups=GROUPS,
                    ins=[ot_ch[qt].opt()],
                    outs=[og_ch[qt].opt()],
                )

            proj_rhs = {}

            def emit_proj_rhs(ch):
                rhs_t = prhs_pool.tile([128, ET, 512], BF16, tag="prhs")
                proj_rhs[ch] = rhs_t
                for kt in range(ET):
                    nc.sync.dma_start(
                        out=rhs_t[:, kt, :],
                        in_=og_ch[ch][kt * 128 : (kt + 1) * 128, :],
                    )

            def emit_proj_mg(ch, mg):
                pps = eps_pool.tile([128, 512], F32, tag="e")
                for kt in range(ET):
                    nc.tensor.matmul(
                        pps[:],
                        lhsT=wproj_sb[:, kt, mg * 128 : (mg + 1) * 128],
                        rhs=proj_rhs[ch][:, kt, :],
                        start=(kt == 0),
                        stop=(kt == ET - 1),
                    )
                ob = outp.tile([128, 512], BF16, tag="ob")
                nc.vector.tensor_scalar(
                    ob[:],
                    pps[:],
                    bias_sb[:, mg : mg + 1],
                    0.0,
                    mybir.AluOpType.add,
                    mybir.AluOpType.max,
                )
                nc.sync.dma_start(
                    out=out_d[mg * 128 : (mg + 1) * 128, ch * 512 : (ch + 1) * 512],
                    in_=ob[:],
                )

            # ---- fused schedule ---------------------------------------
            # lead-in: per n-tile produce K,V (and Q for qt0), and run qt0's
            # attention k-tiles as soon as their K/V exist.
            pv_tiles = {}

            def alloc_pv(qt):
                pv_tiles[qt] = [
                    pvps_pool.tile([65, 512], F32, tag="pv", name=f"pv{qt}_{i}")
                    for i in range(4)
                ]

            alloc_pv(0)
            for nt in range(NT):
                emit_x_load(nt)
                emit_qk(nt, 0, 1)  # k pair 0
                emit_qk(nt, 1, 1)  # k pair 1
                for m in range(4):
                    emit_v(nt, m)
                if nt == 0:
                    emit_qk(0, 0, 0)  # q pair 0 (qt 0)
                    emit_qk(0, 1, 0)  # q pair 1
                if nt == 1:
                    emit_qk(1, 0, 0)  # q for qt 1 (the qt loop emits qt+1)
                    emit_qk(1, 1, 0)
                for kt in range(4 * nt, 4 * nt + 4):
                    attn_slot(0, kt, pv_tiles[0])

            # weights for the projection tail load behind the lead-in
            for kt in range(ET):
                nc.sync.dma_start(out=wproj_sb[:, kt, :], in_=wproj_d[kt])
            nc.sync.dma_start(
                out=bias_sb, in_=bias_d[:].rearrange("(g p) -> p g", p=128)
            )

            emit_norm(0, pv_tiles[0])
            emit_ag(0)

            for qt in range(1, NT):
                alloc_pv(qt)
                for kt in range(KT):
                    attn_slot(qt, kt, pv_tiles[qt])
                    if kt == 4 and qt < NT - 1:
                        emit_qk(qt + 1, 0, 0)  # q for next qt
                    if kt == 6 and qt < NT - 1:
                        emit_qk(qt + 1, 1, 0)
                    if kt == 9:
                        emit_proj_rhs(qt - 1)
                    if kt == 11:
                        emit_proj_mg(qt - 1, 0)
                    if kt == 13:
                        emit_proj_mg(qt - 1, 1)
                emit_norm(qt, pv_tiles[qt])
                emit_ag(qt)
            emit_proj_rhs(NT - 1)
            emit_proj_mg(NT - 1, 0)
            emit_proj_mg(NT - 1, 1)

    nc.compile()
    return nc


def _get_nc():
    global _CACHED_NC
    if _CACHED_NC is None:
        _CACHED_NC = _build()
    return _CACHED_NC


def _prep_inputs(x, w_qkv, w_proj, b_proj):
    """Shard + relayout the full inputs for the 8 cores."""
    x = np.asarray(x, dtype=np.float32)
    w_qkv = np.asarray(w_qkv, dtype=np.float32)
    w_proj = np.asarray(w_proj, dtype=np.float32)
    b_proj = np.asarray(b_proj, dtype=np.float32)

    # x^T per batch: [E, N] -> tiles [ET, 128, N], fp16
    xts = [
        np.ascontiguousarray(x[b].T).reshape(ET, 128, N).astype(np.float16)
        for b in range(B)
    ]
    # w_qkv rows are (h, d, qkv)-interleaved with qkv innermost
    wr = w_qkv.reshape(H, D, 3, E)
    # fold the post-softmax 1/sqrt(E) scaling into w_proj
    wp = w_proj / np.sqrt(E).astype(np.float32)

    wqkv_shards, wproj_shards, bias_shards = [], [], []
    for r in range(4):
        heads = range(4 * r, 4 * r + 4)
        qrows = np.concatenate([wr[h, :, 0, :] for h in heads], 0)  # [256, E]
        krows = np.concatenate([wr[h, :, 1, :] for h in heads], 0)
        vrows = np.concatenate([wr[h, :, 2, :] for h in heads], 0)
        w_core = np.concatenate([qrows, krows, vrows], 0)  # [768, E]
        wqkv_shards.append(
            np.ascontiguousarray(w_core.T).reshape(ET, 128, QKV_F).astype(
                np.float16
            )
        )
        wproj_shards.append(
            np.ascontiguousarray(wp[r * FC : (r + 1) * FC, :].T)
            .reshape(ET, 128, FC)
            .astype(ml_dtypes.bfloat16)
        )
        bias_shards.append(np.ascontiguousarray(b_proj[r * FC : (r + 1) * FC]))

    in_maps = []
    for c in range(NCORES):
        b, r = c // 4, c % 4
        in_maps.append(
            {
                "xt": xts[b],
                "wqkvt": wqkv_shards[r],
                "wprojt": wproj_shards[r],
                "bias": bias_shards[r],
            }
        )
    return in_maps


def kernel(x, w_qkv, w_proj, b_proj):
    global LAST_EXEC_NS, LAST_RESULTS
    nc = _get_nc()
    in_maps = _prep_inputs(x, w_qkv, w_proj, b_proj)
    trace = bool(int(os.environ.get("BASS_KERNEL_TRACE", "0")))
    res = run_bass_kernel_spmd(
        nc, in_maps, list(range(NCORES)), trace=trace
    )
    LAST_EXEC_NS = res.exec_time_ns
    LAST_RESULTS = res

    out = np.empty((B, N, E), dtype=np.float32)
    for g in range(B):
        pt = np.concatenate(
            [
                res.results[4 * g + r]["out"].astype(np.float32)
                for r in range(4)
            ],
            axis=0,
        )  # [1024 f, 2048 n]
        out[g] = pt.T
    return out


# revision 11
# speedup vs baseline: 1.3254x; 1.0376x over previous
"""Distributed multi-head attention kernel for Trainium2 (8 NeuronCores).

Reference computation (EMBED=1024, HEADS=16, b=2, n=2048):
    qkv = x @ w_qkv.T                       -> [b, n, h, d, 3] (qkv innermost)
    q, k, v per head; energy = q @ k^T
    att = softmax(energy, -1) / sqrt(1024)
    out = att @ v -> [b, n, 1024]
    relu(out @ w_proj.T + b_proj)

Sharding: 2-way data parallel over batch x 4-way tensor parallel over heads.
Core c handles batch c//4, heads [4*(c%4) .. 4*(c%4)+3].  After attention,
each 4-core batch group AllGathers the per-core attention output features
and every core computes a 256-feature slice of the output projection.

v2 design (vs the fp32r v1 at 462us):
  * fp16 for the q/k path (x, w_qkv, q, k): 1 cycle/col on the PE vs the
    ~1.5-3x passes fp32/fp32r matmuls cost; rel err ~3e-3 (validated on host).
  * bf16 for everything softmax-onward (exp, v, att out, w_proj, AllGather,
    final output): exp needs bf16 range (values up to e^+45).
  * Single fused pipeline: K/V production interleaves with qt=0's attention
    (kt follows nt availability), Q[qt] and proj[qt-1] are emitted inside
    later qt's kt loop as PE gap fillers.  The ACT engine (exp, ~143us of
    [128,1024] activations) is the target critical path; the PE stream is
    kept dense so HAM stays warm.
  * PSUM budget (8 banks): energy pool [128,1024]f32 x2 (4 banks, shared
    with Q/K/V/proj chunk borrows), PV accumulators [65,512]f32 x4 (4 banks,
    live across each qt's kt loop; row 64 = ones-column softmax denominator).
  * Softmax normalization: denominators scatter-DMA'd to [128,8] so the DVE
    reciprocal runs 128-wide (v1 ran it 64x redundant on a broadcast tile:
    53us of DVE), then broadcast back over 64 partitions for one multiply.
  * AllGather in bf16 (halves wire bytes), one per qt (4 total).
"""

import os
import sys
import types

sys.path.insert(0, "/opt/trn_rl_repo")

import numpy as np
import ml_dtypes


def _install_ntff_shim():
    """The agent image's antenv lacks axon_hooks; recreate it so
    run_bass_kernel_spmd(trace=True) can capture NTFF profiles."""
    try:
        import antenv.axon_hooks  # noqa: F401
        return
    except ImportError:
        pass
    try:
        import antenv
        from trn_agent_boot.trn_boot import _ntff_profile_via_ctypes
    except ImportError:
        return
    mod = types.ModuleType("antenv.axon_hooks")
    _hook = [None]
    mod.set_axon_ntff_profile_hook = lambda h: _hook.__setitem__(0, h)
    mod.get_axon_ntff_profile_hook = lambda: _hook[0]
    sys.modules["antenv.axon_hooks"] = mod
    antenv.axon_hooks = mod
    mod.set_axon_ntff_profile_hook(
        _ntff_profile_via_ctypes("/opt/axon/libaxon_pjrt.so")
    )


_install_ntff_shim()

import concourse.bacc as bacc
import concourse.bass as bass
import concourse.tile as tile
from concourse import mybir
from concourse.bass_utils import run_bass_kernel_spmd

B, N, E, H, D = 2, 2048, 1024, 16, 64
NCORES = 8
GROUPS = [[0, 1, 2, 3], [4, 5, 6, 7]]
HPC = H // 4            # heads per core = 4
FC = HPC * D            # attention-output features per core = 256
QKV_F = 3 * FC          # qkv features per core = 768
ET = E // 128           # 8 k-tiles over the embed dim
NT = N // 512           # 4 n-tiles of 512
KT = N // 128           # 16 k-tiles of 128 over sequence
F32 = mybir.dt.float32
F16 = mybir.dt.float16
BF16 = mybir.dt.bfloat16

LAST_EXEC_NS = None
LAST_RESULTS = None

_CACHED_NC = None


def _build():
    nc = bacc.Bacc("TRN2", target_bir_lowering=False, num_devices=NCORES)

    xt_d = nc.dram_tensor("xt", [ET, 128, N], F16, kind="ExternalInput")
    wqkv_d = nc.dram_tensor("wqkvt", [ET, 128, QKV_F], F16, kind="ExternalInput")
    wproj_d = nc.dram_tensor("wprojt", [ET, 128, FC], BF16, kind="ExternalInput")
    bias_d = nc.dram_tensor("bias", [FC], F32, kind="ExternalInput")
    out_d = nc.dram_tensor("out", [FC, N], BF16, kind="ExternalOutput")

    with tile.TileContext(nc) as tc:
        with (
            tc.tile_pool(name="persist", bufs=1) as persist,
            tc.tile_pool(name="dram", bufs=1, space="DRAM") as dram,
            tc.tile_pool(name="xtp", bufs=NT) as xtp,
            tc.tile_pool(name="eps", bufs=2, space="PSUM") as eps_pool,
            tc.tile_pool(name="pvps", bufs=4, space="PSUM") as pvps_pool,
            tc.tile_pool(name="expp", bufs=4) as expp,
            tc.tile_pool(name="normp", bufs=2) as normp,
            tc.tile_pool(name="prhs", bufs=2) as prhs_pool,
            tc.tile_pool(name="outp", bufs=2) as outp,
        ):
            # ---- persistent SBUF tensors -------------------------------
            wqkv_sb = persist.tile([128, ET, QKV_F], F16)
            nc.sync.dma_start(
                out=wqkv_sb[:], in_=wqkv_d[:].rearrange("k p f -> p k f")
            )
            wproj_sb = persist.tile([128, ET, FC], BF16)
            bias_sb = persist.tile([128, 2], F32)

            # tiny warm-up AllGather: absorbs the first-collective rendezvous
            # / ncfw cold cost while the lead-in computes.
            warm_in = dram.tile([1, 64], BF16, name="warm_in")
            warm_out = dram.tile([4, 64], BF16, name="warm_out")
            nc.gpsimd.collective_compute(
                "AllGather",
                mybir.AluOpType.bypass,
                replica_groups=GROUPS,
                ins=[warm_in.opt()],
                outs=[warm_out.opt()],
            )

            # q/k features of head pair p (2 heads x 64d) on partitions
            qt_sb = persist.tile([128, 2, N], F16)
            kt_sb = persist.tile([128, 2, N], F16)
            # v in [n, d] layout + a ones column per head: slot = [64 v | 1]
            v_sb = persist.tile([128, KT, HPC, 65], BF16)
            ones_col = nc.const_aps.tensor(1.0, [128, KT, HPC, 1], F32)
            nc.vector.tensor_copy(v_sb[:, :, :, 64:65], ones_col)

            # DRAM bounce buffers
            ot_ch = [dram.tile([FC, 512], BF16, name=f"ot{i}") for i in range(NT)]
            og_ch = [
                dram.tile([4 * FC, 512], BF16, name=f"og{i}") for i in range(NT)
            ]

            xts = []

            # ---- emitters ---------------------------------------------
            def emit_x_load(nt):
                xt_t = xtp.tile([128, ET, 512], F16, tag="xt")
                xts.append(xt_t)
                nc.sync.dma_start(
                    out=xt_t[:],
                    in_=xt_d[:, :, nt * 512 : (nt + 1) * 512].rearrange(
                        "k p n -> p k n"
                    ),
                )

            def emit_qk(nt, pair, which):
                # which: 0 -> q, 1 -> k
                ps = eps_pool.tile([128, 512], F32, tag="e")
                off = which * 256 + pair * 128
                for kt in range(ET):
                    nc.tensor.matmul(
                        ps[:],
                        lhsT=wqkv_sb[:, kt, off : off + 128],
                        rhs=xts[nt][:, kt, :],
                        start=(kt == 0),
                        stop=(kt == ET - 1),
                    )
                dst = qt_sb if which == 0 else kt_sb
                nc.vector.tensor_copy(
                    dst[:, pair, nt * 512 : (nt + 1) * 512], ps[:]
                )

            def emit_v(nt, m):
                ps = eps_pool.tile([128, FC], F32, tag="e")
                for kt in range(ET):
                    nc.tensor.matmul(
                        ps[:],
                        lhsT=xts[nt][:, kt, m * 128 : (m + 1) * 128],
                        rhs=wqkv_sb[:, kt, 512:768],
                        start=(kt == 0),
                        stop=(kt == ET - 1),
                    )
                nc.vector.tensor_copy(
                    v_sb[:, nt * 4 + m, :, 0:64],
                    ps[:].rearrange("p (h d) -> p h d", h=HPC),
                )

            def attn_slot(qt, kt, pvt):
                # pvt: list of 4 per-head PV psum accumulators [65, 512]
                q_sl = slice(qt * 512, (qt + 1) * 512)
                exps = []
                for pair in range(2):
                    ep = eps_pool.tile([128, 1024], F32, tag="e")
                    for s in range(2):
                        d_sl = slice(s * 64, (s + 1) * 64)
                        nc.tensor.matmul(
                            ep[:, s * 512 : (s + 1) * 512],
                            lhsT=kt_sb[d_sl, pair, kt * 128 : (kt + 1) * 128],
                            rhs=qt_sb[d_sl, pair, q_sl],
                            start=True,
                            stop=True,
                        )
                    ex = expp.tile([128, 1024], BF16, tag="exp")
                    nc.scalar.activation(
                        ex[:], ep[:], mybir.ActivationFunctionType.Exp
                    )
                    exps.append(ex)
                for pair in range(2):
                    for s in range(2):
                        nc.tensor.matmul(
                            pvt[2 * pair + s][0:65, :],
                            lhsT=v_sb[:, kt, 2 * pair + s, :],
                            rhs=exps[pair][:, s * 512 : (s + 1) * 512],
                            start=(kt == 0),
                            stop=(kt == KT - 1),
                        )

            def emit_norm(qt, pvt):
                # normalize out^T[d, q] by 1/denominator[q] and store to the
                # AllGather input chunk.
                # 1) evacuate PV psum to SBUF immediately so the psum banks
                #    free up for the next qt's accumulators
                den_sb = normp.tile([1, 2048], F32, tag="den_sb")
                pv_sb = normp.tile([64, 2048], F32, tag="pv_sb")
                for i in range(4):
                    c_sl = slice(i * 512, (i + 1) * 512)
                    nc.vector.tensor_copy(den_sb[:, c_sl], pvt[i][64:65, :])
                    nc.vector.tensor_copy(pv_sb[:, c_sl], pvt[i][0:64, :])
                # 2) reciprocal on a [128, 16] scatter so the DVE is
                #    full-width (one DRAM bounce round-trip for all 4 heads)
                den_d = dram.tile([1, 2048], F32, tag="den", bufs=2)
                nc.sync.dma_start(out=den_d[:], in_=den_sb[:])
                den_sc = normp.tile([128, 16], F32, tag="den_sc")
                nc.sync.dma_start(
                    out=den_sc[:],
                    in_=bass.AP(
                        tensor=den_d.tensor,
                        offset=den_d.offset,
                        ap=[[16, 128], [1, 16]],
                    ),
                )
                rec_sc = normp.tile([128, 16], F32, tag="rec_sc")
                nc.vector.reciprocal(rec_sc[:], den_sc[:])
                rec_d = dram.tile([1, 2048], F32, tag="rec", bufs=2)
                nc.sync.dma_start(
                    out=bass.AP(
                        tensor=rec_d.tensor,
                        offset=rec_d.offset,
                        ap=[[16, 128], [1, 16]],
                    ),
                    in_=rec_sc[:],
                )
                rep = normp.tile([64, 2048], F32, tag="rep")
                nc.sync.dma_start(
                    out=rep[:],
                    in_=bass.AP(
                        tensor=rec_d.tensor,
                        offset=rec_d.offset,
                        ap=[[0, 64], [1, 2048]],
                    ),
                )
                # 3) normalize + store: one DMA per pair into the AG chunk
                o_sb = normp.tile([64, 2048], BF16, tag="o")
                nc.vector.tensor_mul(o_sb[:], pv_sb[:], rep[:])
                for pair in range(2):
                    nc.sync.dma_start(
                        out=ot_ch[qt][
                            2 * pair * 64 : 2 * (pair + 1) * 64, :
                        ].rearrange("(s dd) q -> dd s q", s=2),
                        in_=o_sb[:, pair * 1024 : (pair + 1) * 1024].rearrange(
                            "dd (s q) -> dd s q", s=2
                        ),
                    )

            def emit_ag(qt):
                nc.gpsimd.collective_compute(
                    "AllGather",
                    mybir.AluOpType.bypass,
                    replica_groups=GROUPS,
                    ins=[ot_ch[qt].opt()],
                    outs=[og_ch[qt].opt()],
                )

            proj_rhs = {}

            def emit_proj_rhs(ch):
                rhs_t = prhs_pool.tile([128, ET, 512], BF16, tag="prhs")
                proj_rhs[ch] = rhs_t
                # gpsimd queue: this DMA waits on the AllGather; keep that
                # wait off the sync queue so norm-chain DMAs never stall
                nc.gpsimd.dma_start(
                    out=rhs_t[:],
                    in_=og_ch[ch][:].rearrange("(k p) n -> p k n", p=128),
                )

            def emit_proj_mg(ch, mg):
                pps = eps_pool.tile([128, 512], F32, tag="e")
                for kt in range(ET):
                    nc.tensor.matmul(
                        pps[:],
                        lhsT=wproj_sb[:, kt, mg * 128 : (mg + 1) * 128],
                        rhs=proj_rhs[ch][:, kt, :],
                        start=(kt == 0),
                        stop=(kt == ET - 1),
                    )
                ob = outp.tile([128, 512], BF16, tag="ob")
                nc.vector.tensor_scalar(
                    ob[:],
                    pps[:],
                    bias_sb[:, mg : mg + 1],
                    0.0,
                    mybir.AluOpType.add,
                    mybir.AluOpType.max,
                )
                nc.sync.dma_start(
                    out=out_d[mg * 128 : (mg + 1) * 128, ch * 512 : (ch + 1) * 512],
                    in_=ob[:],
                )

            # ---- fused schedule ---------------------------------------
            # lead-in: per n-tile produce K,V (and Q for qt0), and run qt0's
            # attention k-tiles as soon as their K/V exist.
            pv_tiles = {}

            def alloc_pv(qt):
                pv_tiles[qt] = [
                    pvps_pool.tile([65, 512], F32, tag="pv", name=f"pv{qt}_{i}")
                    for i in range(4)
                ]

            alloc_pv(0)
            for nt in range(NT):
                emit_x_load(nt)
                emit_qk(nt, 0, 1)  # k pair 0
                emit_qk(nt, 1, 1)  # k pair 1
                for m in range(4):
                    emit_v(nt, m)
                if nt == 0:
                    emit_qk(0, 0, 0)  # q pair 0 (qt 0)
                    emit_qk(0, 1, 0)  # q pair 1
                if nt == 1:
                    emit_qk(1, 0, 0)  # q for qt 1 (the qt loop emits qt+1)
                    emit_qk(1, 1, 0)
                for kt in range(4 * nt, 4 * nt + 4):
                    attn_slot(0, kt, pv_tiles[0])

            # weights for the projection tail load behind the lead-in
            nc.sync.dma_start(
                out=wproj_sb[:], in_=wproj_d[:].rearrange("k p f -> p k f")
            )
            nc.sync.dma_start(
                out=bias_sb, in_=bias_d[:].rearrange("(g p) -> p g", p=128)
            )

            emit_norm(0, pv_tiles[0])
            emit_ag(0)

            # proj chunks run with a TWO-qt lag so their matmuls never reach
            # the PE queue head before their AllGather has completed (a proj
            # matmul waiting on a collective would head-of-line-block every
            # later PE instruction).
            for qt in range(1, NT):
                alloc_pv(qt)
                for kt in range(KT):
                    attn_slot(qt, kt, pv_tiles[qt])
                    if kt == 4 and qt < NT - 1:
                        emit_qk(qt + 1, 0, 0)  # q for next qt
                    if kt == 6 and qt < NT - 1:
                        emit_qk(qt + 1, 1, 0)
                    if qt >= 2:
                        # qt==2 runs proj(0); qt==3 runs proj(1) and proj(2)
                        if kt == 1:
                            emit_proj_rhs(qt - 2)
                        if kt == 3:
                            emit_proj_mg(qt - 2, 0)
                        if kt == 5:
                            emit_proj_mg(qt - 2, 1)
                        if qt == NT - 1:
                            if kt == 9:
                                emit_proj_rhs(qt - 1)
                            if kt == 11:
                                emit_proj_mg(qt - 1, 0)
                            if kt == 13:
                                emit_proj_mg(qt - 1, 1)
                emit_norm(qt, pv_tiles[qt])
                emit_ag(qt)
            emit_proj_rhs(NT - 1)
            emit_proj_mg(NT - 1, 0)
            emit_proj_mg(NT - 1, 1)

    nc.compile()
    return nc


def _get_nc():
    global _CACHED_NC
    if _CACHED_NC is None:
        _CACHED_NC = _build()
    return _CACHED_NC


def _prep_inputs(x, w_qkv, w_proj, b_proj):
    """Shard + relayout the full inputs for the 8 cores."""
    x = np.asarray(x, dtype=np.float32)
    w_qkv = np.asarray(w_qkv, dtype=np.float32)
    w_proj = np.asarray(w_proj, dtype=np.float32)
    b_proj = np.asarray(b_proj, dtype=np.float32)

    # x^T per batch: [E, N] -> tiles [ET, 128, N], fp16
    xts = [
        np.ascontiguousarray(x[b].T).reshape(ET, 128, N).astype(np.float16)
        for b in range(B)
    ]
    # w_qkv rows are (h, d, qkv)-interleaved with qkv innermost
    wr = w_qkv.reshape(H, D, 3, E)
    # fold the post-softmax 1/sqrt(E) scaling into w_proj
    wp = w_proj / np.sqrt(E).astype(np.float32)

    wqkv_shards, wproj_shards, bias_shards = [], [], []
    for r in range(4):
        heads = range(4 * r, 4 * r + 4)
        qrows = np.concatenate([wr[h, :, 0, :] for h in heads], 0)  # [256, E]
        krows = np.concatenate([wr[h, :, 1, :] for h in heads], 0)
        vrows = np.concatenate([wr[h, :, 2, :] for h in heads], 0)
        w_core = np.concatenate([qrows, krows, vrows], 0)  # [768, E]
        wqkv_shards.append(
            np.ascontiguousarray(w_core.T).reshape(ET, 128, QKV_F).astype(
                np.float16
            )
        )
        wproj_shards.append(
            np.ascontiguousarray(wp[r * FC : (r + 1) * FC, :].T)
            .reshape(ET, 128, FC)
            .astype(ml_dtypes.bfloat16)
        )
        bias_shards.append(np.ascontiguousarray(b_proj[r * FC : (r + 1) * FC]))

    in_maps = []
    for c in range(NCORES):
        b, r = c // 4, c % 4
        in_maps.append(
            {
                "xt": xts[b],
                "wqkvt": wqkv_shards[r],
                "wprojt": wproj_shards[r],
                "bias": bias_shards[r],
            }
        )
    return in_maps


def kernel(x, w_qkv, w_proj, b_proj):
    global LAST_EXEC_NS, LAST_RESULTS
    nc = _get_nc()
    in_maps = _prep_inputs(x, w_qkv, w_proj, b_proj)
    trace = bool(int(os.environ.get("BASS_KERNEL_TRACE", "0")))
    res = run_bass_kernel_spmd(
        nc, in_maps, list(range(NCORES)), trace=trace
    )
    LAST_EXEC_NS = res.exec_time_ns
    LAST_RESULTS = res

    out = np.empty((B, N, E), dtype=np.float32)
    for g in range(B):
        pt = np.concatenate(
            [
                res.results[4 * g + r]["out"].astype(np.float32)
                for r in range(4)
            ],
            axis=0,
        )  # [1024 f, 2048 n]
        out[g] = pt.T
    return out


# revision 15
# speedup vs baseline: 1.5318x; 1.1557x over previous
"""Distributed multi-head attention kernel for Trainium2 (8 NeuronCores).

Reference computation (EMBED=1024, HEADS=16, b=2, n=2048):
    qkv = x @ w_qkv.T                       -> [b, n, h, d, 3] (qkv innermost)
    q, k, v per head; energy = q @ k^T
    att = softmax(energy, -1) / sqrt(1024)
    out = att @ v -> [b, n, 1024]
    relu(out @ w_proj.T + b_proj)

Sharding: 2-way data parallel over batch x 4-way tensor parallel over heads.
Core c handles batch c//4, heads [4*(c%4) .. 4*(c%4)+3].  After attention,
each 4-core batch group AllGathers the per-core attention output features
and every core computes a 256-feature slice of the output projection.

v2 design (vs the fp32r v1 at 462us):
  * fp16 for the q/k path (x, w_qkv, q, k): 1 cycle/col on the PE vs the
    ~1.5-3x passes fp32/fp32r matmuls cost; rel err ~3e-3 (validated on host).
  * bf16 for everything softmax-onward (exp, v, att out, w_proj, AllGather,
    final output): exp needs bf16 range (values up to e^+45).
  * Single fused pipeline: K/V production interleaves with qt=0's attention
    (kt follows nt availability), Q[qt] and proj[qt-1] are emitted inside
    later qt's kt loop as PE gap fillers.  The ACT engine (exp, ~143us of
    [128,1024] activations) is the target critical path; the PE stream is
    kept dense so HAM stays warm.
  * PSUM budget (8 banks): energy pool [128,1024]f32 x2 (4 banks, shared
    with Q/K/V/proj chunk borrows), PV accumulators [65,512]f32 x4 (4 banks,
    live across each qt's kt loop; row 64 = ones-column softmax denominator).
  * Softmax normalization: denominators scatter-DMA'd to [128,8] so the DVE
    reciprocal runs 128-wide (v1 ran it 64x redundant on a broadcast tile:
    53us of DVE), then broadcast back over 64 partitions for one multiply.
  * AllGather in bf16 (halves wire bytes), one per qt (4 total).
"""

import os
import sys
import types

sys.path.insert(0, "/opt/trn_rl_repo")

import numpy as np
import ml_dtypes


def _install_ntff_shim():
    """The agent image's antenv lacks axon_hooks; recreate it so
    run_bass_kernel_spmd(trace=True) can capture NTFF profiles."""
    try:
        import antenv.axon_hooks  # noqa: F401
        return
    except ImportError:
        pass
    try:
        import antenv
        from trn_agent_boot.trn_boot import _ntff_profile_via_ctypes
    except ImportError:
        return
    mod = types.ModuleType("antenv.axon_hooks")
    _hook = [None]
    mod.set_axon_ntff_profile_hook = lambda h: _hook.__setitem__(0, h)
    mod.get_axon_ntff_profile_hook = lambda: _hook[0]
    sys.modules["antenv.axon_hooks"] = mod
    antenv.axon_hooks = mod
    mod.set_axon_ntff_profile_hook(
        _ntff_profile_via_ctypes("/opt/axon/libaxon_pjrt.so")
    )


_install_ntff_shim()

import concourse.bacc as bacc
import concourse.bass as bass
import concourse.tile as tile
from concourse import mybir
from concourse.bass_utils import run_bass_kernel_spmd

B, N, E, H, D = 2, 2048, 1024, 16, 64
NCORES = 8
GROUPS = [[0, 1, 2, 3], [4, 5, 6, 7]]
HPC = H // 4            # heads per core = 4
FC = HPC * D            # attention-output features per core = 256
QKV_F = 3 * FC          # qkv features per core = 768
ET = E // 128           # 8 k-tiles over the embed dim
NT = N // 512           # 4 n-tiles of 512
KT = N // 128           # 16 k-tiles of 128 over sequence
F32 = mybir.dt.float32
F16 = mybir.dt.float16
BF16 = mybir.dt.bfloat16

LAST_EXEC_NS = None
LAST_RESULTS = None

_CACHED_NC = None


def _build():
    nc = bacc.Bacc("TRN2", target_bir_lowering=False, num_devices=NCORES)

    xt_d = nc.dram_tensor("xt", [ET, 128, N], F16, kind="ExternalInput")
    wqkv_d = nc.dram_tensor("wqkvt", [ET, 128, QKV_F], F16, kind="ExternalInput")
    wproj_d = nc.dram_tensor("wprojt", [ET, 128, FC], BF16, kind="ExternalInput")
    bias_d = nc.dram_tensor("bias", [FC], F32, kind="ExternalInput")
    out_d = nc.dram_tensor("out", [FC, N], BF16, kind="ExternalOutput")

    with tile.TileContext(nc) as tc:
        with (
            tc.tile_pool(name="persist", bufs=1) as persist,
            tc.tile_pool(name="dram", bufs=1, space="DRAM") as dram,
            tc.tile_pool(name="xtp", bufs=NT) as xtp,
            tc.tile_pool(name="eps", bufs=2, space="PSUM") as eps_pool,
            tc.tile_pool(name="pvps", bufs=4, space="PSUM") as pvps_pool,
            tc.tile_pool(name="expp", bufs=4) as expp,
            tc.tile_pool(name="normp", bufs=2) as normp,
            tc.tile_pool(name="prhs", bufs=2) as prhs_pool,
            tc.tile_pool(name="outp", bufs=2) as outp,
        ):
            # ---- persistent SBUF tensors -------------------------------
            wqkv_sb = persist.tile([128, ET, QKV_F], F16)
            wproj_sb = persist.tile([128, ET, FC], BF16)
            bias_sb = persist.tile([128, 2], F32)

            # tiny warm-up AllGather: absorbs the first-collective rendezvous
            # / ncfw cold cost while the lead-in computes.
            warm_in = dram.tile([1, 64], BF16, name="warm_in")
            warm_out = dram.tile([4, 64], BF16, name="warm_out")
            nc.gpsimd.collective_compute(
                "AllGather",
                mybir.AluOpType.bypass,
                replica_groups=GROUPS,
                ins=[warm_in.opt()],
                outs=[warm_out.opt()],
            )

            # q/k features of head pair p (2 heads x 64d) on partitions
            qt_sb = persist.tile([128, 2, N], F16)
            kt_sb = persist.tile([128, 2, N], F16)
            # v in [n, d] layout + a ones column per head: slot = [64 v | 1]
            v_sb = persist.tile([128, KT, HPC, 65], BF16)
            ones_col = nc.const_aps.tensor(1.0, [128, KT, HPC, 1], F32)
            nc.vector.tensor_copy(v_sb[:, :, :, 64:65], ones_col)

            # DRAM bounce buffers
            ot_ch = [dram.tile([FC, 512], BF16, name=f"ot{i}") for i in range(NT)]
            og_ch = [
                dram.tile([4 * FC, 512], BF16, name=f"og{i}") for i in range(NT)
            ]

            xts = []

            # ---- emitters ---------------------------------------------
            def emit_x_load(nt):
                xt_t = xtp.tile([128, ET, 512], F16, tag="xt")
                xts.append(xt_t)
                nc.sync.dma_start(
                    out=xt_t[:],
                    in_=xt_d[:, :, nt * 512 : (nt + 1) * 512].rearrange(
                        "k p n -> p k n"
                    ),
                )

            def emit_qk(nt, pair, which):
                # which: 0 -> q, 1 -> k
                ps = eps_pool.tile([128, 512], F32, tag="e")
                off = which * 256 + pair * 128
                for kt in range(ET):
                    nc.tensor.matmul(
                        ps[:],
                        lhsT=wqkv_sb[:, kt, off : off + 128],
                        rhs=xts[nt][:, kt, :],
                        start=(kt == 0),
                        stop=(kt == ET - 1),
                    )
                dst = qt_sb if which == 0 else kt_sb
                nc.vector.tensor_copy(
                    dst[:, pair, nt * 512 : (nt + 1) * 512], ps[:]
                )

            def emit_v(nt, m):
                ps = eps_pool.tile([128, FC], F32, tag="e")
                for kt in range(ET):
                    nc.tensor.matmul(
                        ps[:],
                        lhsT=xts[nt][:, kt, m * 128 : (m + 1) * 128],
                        rhs=wqkv_sb[:, kt, 512:768],
                        start=(kt == 0),
                        stop=(kt == ET - 1),
                    )
                nc.vector.tensor_copy(
                    v_sb[:, nt * 4 + m, :, 0:64],
                    ps[:].rearrange("p (h d) -> p h d", h=HPC),
                )

            def attn_slot(qt, kt, pvt):
                # pvt: list of 4 per-head PV psum accumulators [65, 512]
                q_sl = slice(qt * 512, (qt + 1) * 512)
                exps = []
                for pair in range(2):
                    ep = eps_pool.tile([128, 1024], F32, tag="e")
                    for s in range(2):
                        d_sl = slice(s * 64, (s + 1) * 64)
                        nc.tensor.matmul(
                            ep[:, s * 512 : (s + 1) * 512],
                            lhsT=kt_sb[d_sl, pair, kt * 128 : (kt + 1) * 128],
                            rhs=qt_sb[d_sl, pair, q_sl],
                            start=True,
                            stop=True,
                        )
                    ex = expp.tile([128, 1024], BF16, tag="exp")
                    nc.scalar.activation(
                        ex[:], ep[:], mybir.ActivationFunctionType.Exp
                    )
                    exps.append(ex)
                for pair in range(2):
                    for s in range(2):
                        nc.tensor.matmul(
                            pvt[2 * pair + s][0:65, :],
                            lhsT=v_sb[:, kt, 2 * pair + s, :],
                            rhs=exps[pair][:, s * 512 : (s + 1) * 512],
                            start=(kt == 0),
                            stop=(kt == KT - 1),
                        )

            def emit_norm(qt, pvt):
                # normalize out^T[d, q] by 1/denominator[q] and store to the
                # AllGather input chunk.
                # 1) evacuate PV psum to SBUF immediately so the psum banks
                #    free up for the next qt's accumulators.  denominators go
                #    through the DVE (heads the DMA chain ASAP); the big pv
                #    copies run on the otherwise-idle ACT engine in parallel.
                den_sb = normp.tile([1, 2048], F32, tag="den_sb")
                pv_sb = normp.tile([64, 2048], F32, tag="pv_sb")
                for i in range(4):
                    c_sl = slice(i * 512, (i + 1) * 512)
                    nc.vector.tensor_copy(den_sb[:, c_sl], pvt[i][64:65, :])
                    nc.scalar.copy(pv_sb[:, c_sl], pvt[i][0:64, :])
                # 2) reciprocal on a [128, 16] scatter so the DVE is
                #    full-width (one DRAM bounce round-trip for all 4 heads)
                den_d = dram.tile([1, 2048], F32, tag="den", bufs=2)
                nc.sync.dma_start(out=den_d[:], in_=den_sb[:])
                den_sc = normp.tile([128, 16], F32, tag="den_sc")
                nc.sync.dma_start(
                    out=den_sc[:],
                    in_=bass.AP(
                        tensor=den_d.tensor,
                        offset=den_d.offset,
                        ap=[[16, 128], [1, 16]],
                    ),
                )
                rec_sc = normp.tile([128, 16], F32, tag="rec_sc")
                nc.vector.reciprocal(rec_sc[:], den_sc[:])
                rec_d = dram.tile([1, 2048], F32, tag="rec", bufs=2)
                nc.sync.dma_start(
                    out=bass.AP(
                        tensor=rec_d.tensor,
                        offset=rec_d.offset,
                        ap=[[16, 128], [1, 16]],
                    ),
                    in_=rec_sc[:],
                )
                rep = normp.tile([64, 2048], F32, tag="rep")
                nc.sync.dma_start(
                    out=rep[:],
                    in_=bass.AP(
                        tensor=rec_d.tensor,
                        offset=rec_d.offset,
                        ap=[[0, 64], [1, 2048]],
                    ),
                )
                # 3) normalize + store: one DMA per pair into the AG chunk
                o_sb = normp.tile([64, 2048], BF16, tag="o")
                nc.vector.tensor_mul(o_sb[:], pv_sb[:], rep[:])
                for pair in range(2):
                    nc.sync.dma_start(
                        out=ot_ch[qt][
                            2 * pair * 64 : 2 * (pair + 1) * 64, :
                        ].rearrange("(s dd) q -> dd s q", s=2),
                        in_=o_sb[:, pair * 1024 : (pair + 1) * 1024].rearrange(
                            "dd (s q) -> dd s q", s=2
                        ),
                    )

            def emit_ag(qt):
                nc.gpsimd.collective_compute(
                    "AllGather",
                    mybir.AluOpType.bypass,
                    replica_groups=GROUPS,
                    ins=[ot_ch[qt].opt()],
                    outs=[og_ch[qt].opt()],
                )

            proj_rhs = {}

            def emit_proj_rhs(ch):
                rhs_t = prhs_pool.tile([128, ET, 512], BF16, tag="prhs")
                proj_rhs[ch] = rhs_t
                # gpsimd queue: this DMA waits on the AllGather; keep that
                # wait off the sync queue so norm-chain DMAs never stall
                nc.gpsimd.dma_start(
                    out=rhs_t[:],
                    in_=og_ch[ch][:].rearrange("(k p) n -> p k n", p=128),
                )

            def emit_proj_mg(ch, mg):
                pps = eps_pool.tile([128, 512], F32, tag="e")
                for kt in range(ET):
                    nc.tensor.matmul(
                        pps[:],
                        lhsT=wproj_sb[:, kt, mg * 128 : (mg + 1) * 128],
                        rhs=proj_rhs[ch][:, kt, :],
                        start=(kt == 0),
                        stop=(kt == ET - 1),
                    )
                ob = outp.tile([128, 512], BF16, tag="ob")
                nc.vector.tensor_scalar(
                    ob[:],
                    pps[:],
                    bias_sb[:, mg : mg + 1],
                    0.0,
                    mybir.AluOpType.add,
                    mybir.AluOpType.max,
                )
                nc.sync.dma_start(
                    out=out_d[mg * 128 : (mg + 1) * 128, ch * 512 : (ch + 1) * 512],
                    in_=ob[:],
                )

            # ---- fused schedule ---------------------------------------
            # lead-in: per n-tile produce K,V (and Q for qt0), and run qt0's
            # attention k-tiles as soon as their K/V exist.
            pv_tiles = {}

            def alloc_pv(qt):
                pv_tiles[qt] = [
                    pvps_pool.tile([65, 512], F32, tag="pv", name=f"pv{qt}_{i}")
                    for i in range(4)
                ]

            alloc_pv(0)
            for nt in range(NT):
                emit_x_load(nt)
                if nt == 0:
                    # after x(nt0) so the first K matmuls start ASAP
                    nc.sync.dma_start(
                        out=wqkv_sb[:],
                        in_=wqkv_d[:].rearrange("k p f -> p k f"),
                    )
                emit_qk(nt, 0, 1)  # k pair 0
                emit_qk(nt, 1, 1)  # k pair 1
                for m in range(4):
                    emit_v(nt, m)
                if nt == 0:
                    emit_qk(0, 0, 0)  # q pair 0 (qt 0)
                    emit_qk(0, 1, 0)  # q pair 1
                if nt == 1:
                    emit_qk(1, 0, 0)  # q for qt 1 (the qt loop emits qt+1)
                    emit_qk(1, 1, 0)
                for kt in range(4 * nt, 4 * nt + 4):
                    attn_slot(0, kt, pv_tiles[0])

            # weights for the projection tail load behind the lead-in
            nc.sync.dma_start(
                out=wproj_sb[:], in_=wproj_d[:].rearrange("k p f -> p k f")
            )
            nc.sync.dma_start(
                out=bias_sb, in_=bias_d[:].rearrange("(g p) -> p g", p=128)
            )

            emit_norm(0, pv_tiles[0])
            emit_ag(0)

            # proj chunks run with a TWO-qt lag so their matmuls never reach
            # the PE queue head before their AllGather has completed (a proj
            # matmul waiting on a collective would head-of-line-block every
            # later PE instruction).
            for qt in range(1, NT):
                alloc_pv(qt)
                for kt in range(KT):
                    attn_slot(qt, kt, pv_tiles[qt])
                    if kt == 4 and qt < NT - 1:
                        emit_qk(qt + 1, 0, 0)  # q for next qt
                    if kt == 6 and qt < NT - 1:
                        emit_qk(qt + 1, 1, 0)
                    if qt >= 2:
                        # qt==2 runs proj(0); qt==3 runs proj(1) mid-loop.
                        # placement at kt>=8 guarantees the AllGather (done
                        # ~20us into the NEXT qt at worst) has completed, so
                        # the proj matmuls never head-of-line-block the PE.
                        if kt == 8:
                            emit_proj_rhs(qt - 2)
                        if kt == 10:
                            emit_proj_mg(qt - 2, 0)
                        if kt == 12:
                            emit_proj_mg(qt - 2, 1)
                    if qt == NT - 1 and kt == 14:
                        emit_proj_rhs(qt - 1)
                if qt < NT - 1:
                    emit_norm(qt, pv_tiles[qt])
                    emit_ag(qt)
                else:
                    # tail: norm(3)+AG(3) run on ACT/DVE/DMA/cc while the PE
                    # finishes proj(2); proj(3) follows once AG(3) lands.
                    emit_norm(qt, pv_tiles[qt])
                    emit_ag(qt)
                    emit_proj_mg(qt - 1, 0)
                    emit_proj_mg(qt - 1, 1)
            emit_proj_rhs(NT - 1)
            emit_proj_mg(NT - 1, 0)
            emit_proj_mg(NT - 1, 1)

    nc.compile()
    return nc


def _get_nc():
    global _CACHED_NC
    if _CACHED_NC is None:
        _CACHED_NC = _build()
    return _CACHED_NC


def _prep_inputs(x, w_qkv, w_proj, b_proj):
    """Shard + relayout the full inputs for the 8 cores."""
    x = np.asarray(x, dtype=np.float32)
    w_qkv = np.asarray(w_qkv, dtype=np.float32)
    w_proj = np.asarray(w_proj, dtype=np.float32)
    b_proj = np.asarray(b_proj, dtype=np.float32)

    # x^T per batch: [E, N] -> tiles [ET, 128, N], fp16
    xts = [
        np.ascontiguousarray(x[b].T).reshape(ET, 128, N).astype(np.float16)
        for b in range(B)
    ]
    # w_qkv rows are (h, d, qkv)-interleaved with qkv innermost
    wr = w_qkv.reshape(H, D, 3, E)
    # fold the post-softmax 1/sqrt(E) scaling into w_proj
    wp = w_proj / np.sqrt(E).astype(np.float32)

    wqkv_shards, wproj_shards, bias_shards = [], [], []
    for r in range(4):
        heads = range(4 * r, 4 * r + 4)
        qrows = np.concatenate([wr[h, :, 0, :] for h in heads], 0)  # [256, E]
        krows = np.concatenate([wr[h, :, 1, :] for h in heads], 0)
        vrows = np.concatenate([wr[h, :, 2, :] for h in heads], 0)
        w_core = np.concatenate([qrows, krows, vrows], 0)  # [768, E]
        wqkv_shards.append(
            np.ascontiguousarray(w_core.T).reshape(ET, 128, QKV_F).astype(
                np.float16
            )
        )
        wproj_shards.append(
            np.ascontiguousarray(wp[r * FC : (r + 1) * FC, :].T)
            .reshape(ET, 128, FC)
            .astype(ml_dtypes.bfloat16)
        )
        bias_shards.append(np.ascontiguousarray(b_proj[r * FC : (r + 1) * FC]))

    in_maps = []
    for c in range(NCORES):
        b, r = c // 4, c % 4
        in_maps.append(
            {
                "xt": xts[b],
                "wqkvt": wqkv_shards[r],
                "wprojt": wproj_shards[r],
                "bias": bias_shards[r],
            }
        )
    return in_maps


def kernel(x, w_qkv, w_proj, b_proj):
    global LAST_EXEC_NS, LAST_RESULTS
    nc = _get_nc()
    in_maps = _prep_inputs(x, w_qkv, w_proj, b_proj)
    trace = bool(int(os.environ.get("BASS_KERNEL_TRACE", "0")))
    res = run_bass_kernel_spmd(
        nc, in_maps, list(range(NCORES)), trace=trace
    )
    LAST_EXEC_NS = res.exec_time_ns
    LAST_RESULTS = res

    out = np.empty((B, N, E), dtype=np.float32)
    for g in range(B):
        pt = np.concatenate(
            [
                res.results[4 * g + r]["out"].astype(np.float32)
                for r in range(4)
            ],
            axis=0,
        )  # [1024 f, 2048 n]
        out[g] = pt.T
    return out


# revision 23
# speedup vs baseline: 1.5716x; 1.0260x over previous
"""Distributed multi-head attention kernel for Trainium2 (8 NeuronCores).

Reference computation (EMBED=1024, HEADS=16, b=2, n=2048):
    qkv = x @ w_qkv.T                       -> [b, n, h, d, 3] (qkv innermost)
    q, k, v per head; energy = q @ k^T
    att = softmax(energy, -1) / sqrt(1024)
    out = att @ v -> [b, n, 1024]
    relu(out @ w_proj.T + b_proj)

Sharding: 2-way data parallel over batch x 4-way tensor parallel over heads.
Core c handles batch c//4, heads [4*(c%4) .. 4*(c%4)+3].  After attention,
each 4-core batch group AllGathers the per-core attention output features
and every core computes a 256-feature slice of the output projection.

v2 design (vs the fp32r v1 at 462us):
  * fp16 for the q/k path (x, w_qkv, q, k): 1 cycle/col on the PE vs the
    ~1.5-3x passes fp32/fp32r matmuls cost; rel err ~3e-3 (validated on host).
  * bf16 for everything softmax-onward (exp, v, att out, w_proj, AllGather,
    final output): exp needs bf16 range (values up to e^+45).
  * Single fused pipeline: K/V production interleaves with qt=0's attention
    (kt follows nt availability), Q[qt] and proj[qt-1] are emitted inside
    later qt's kt loop as PE gap fillers.  The ACT engine (exp, ~143us of
    [128,1024] activations) is the target critical path; the PE stream is
    kept dense so HAM stays warm.
  * PSUM budget (8 banks): energy pool [128,1024]f32 x2 (4 banks, shared
    with Q/K/V/proj chunk borrows), PV accumulators [65,512]f32 x4 (4 banks,
    live across each qt's kt loop; row 64 = ones-column softmax denominator).
  * Softmax normalization: denominators scatter-DMA'd to [128,8] so the DVE
    reciprocal runs 128-wide (v1 ran it 64x redundant on a broadcast tile:
    53us of DVE), then broadcast back over 64 partitions for one multiply.
  * AllGather in bf16 (halves wire bytes), one per qt (4 total).
"""

import os
import sys
import types

sys.path.insert(0, "/opt/trn_rl_repo")

import numpy as np
import ml_dtypes


def _install_ntff_shim():
    """The agent image's antenv lacks axon_hooks; recreate it so
    run_bass_kernel_spmd(trace=True) can capture NTFF profiles."""
    try:
        import antenv.axon_hooks  # noqa: F401
        return
    except ImportError:
        pass
    try:
        import antenv
        from trn_agent_boot.trn_boot import _ntff_profile_via_ctypes
    except ImportError:
        return
    mod = types.ModuleType("antenv.axon_hooks")
    _hook = [None]
    mod.set_axon_ntff_profile_hook = lambda h: _hook.__setitem__(0, h)
    mod.get_axon_ntff_profile_hook = lambda: _hook[0]
    sys.modules["antenv.axon_hooks"] = mod
    antenv.axon_hooks = mod
    mod.set_axon_ntff_profile_hook(
        _ntff_profile_via_ctypes("/opt/axon/libaxon_pjrt.so")
    )


_install_ntff_shim()

import concourse.bacc as bacc
import concourse.bass as bass
import concourse.tile as tile
from concourse import mybir
from concourse.bass_utils import run_bass_kernel_spmd

B, N, E, H, D = 2, 2048, 1024, 16, 64
NCORES = 8
GROUPS = [[0, 1, 2, 3], [4, 5, 6, 7]]
HPC = H // 4            # heads per core = 4
FC = HPC * D            # attention-output features per core = 256
QKV_F = 3 * FC          # qkv features per core = 768
ET = E // 128           # 8 k-tiles over the embed dim
NT = N // 512           # 4 n-tiles of 512
KT = N // 128           # 16 k-tiles of 128 over sequence
F32 = mybir.dt.float32
F16 = mybir.dt.float16
BF16 = mybir.dt.bfloat16

LAST_EXEC_NS = None
LAST_RESULTS = None

_CACHED_NC = None


def _build():
    nc = bacc.Bacc("TRN2", target_bir_lowering=False, num_devices=NCORES)

    # host-side layouts are pre-transposed so every input load is one fully
    # contiguous DMA burst
    xt_d = nc.dram_tensor("xt", [NT, 128, ET, 512], F16, kind="ExternalInput")
    wqkv_d = nc.dram_tensor("wqkvt", [128, ET, QKV_F], F16, kind="ExternalInput")
    wproj_d = nc.dram_tensor("wprojt", [128, ET, FC], BF16, kind="ExternalInput")
    bias_d = nc.dram_tensor("bias", [FC], F32, kind="ExternalInput")
    out_d = nc.dram_tensor("out", [FC, N], BF16, kind="ExternalOutput")

    with tile.TileContext(nc) as tc:
        with (
            tc.tile_pool(name="persist", bufs=1) as persist,
            tc.tile_pool(name="dram", bufs=1, space="DRAM") as dram,
            tc.tile_pool(name="xtp", bufs=NT) as xtp,
            tc.tile_pool(name="eps", bufs=2, space="PSUM") as eps_pool,
            tc.tile_pool(name="pvps", bufs=4, space="PSUM") as pvps_pool,
            tc.tile_pool(name="expp", bufs=4) as expp,
            tc.tile_pool(name="normp", bufs=2) as normp,
            tc.tile_pool(name="prhs", bufs=2) as prhs_pool,
            tc.tile_pool(name="outp", bufs=2) as outp,
        ):
            # ---- persistent SBUF tensors -------------------------------
            wqkv_sb = persist.tile([128, ET, QKV_F], F16)
            wproj_sb = persist.tile([128, ET, FC], BF16)
            bias_sb = persist.tile([128, 2], F32)

            # tiny warm-up AllGather: absorbs the first-collective rendezvous
            # / ncfw cold cost while the lead-in computes.
            warm_in = dram.tile([1, 64], BF16, name="warm_in")
            warm_out = dram.tile([4, 64], BF16, name="warm_out")
            nc.gpsimd.collective_compute(
                "AllGather",
                mybir.AluOpType.bypass,
                replica_groups=GROUPS,
                ins=[warm_in.opt()],
                outs=[warm_out.opt()],
            )

            # q/k features of head pair p (2 heads x 64d) on partitions
            qt_sb = persist.tile([128, 2, N], F16)
            kt_sb = persist.tile([128, 2, N], F16)
            # v in [n, d] layout + a ones column per head: slot = [64 v | 1]
            v_sb = persist.tile([128, KT, HPC, 65], BF16)
            ones_col = nc.const_aps.tensor(1.0, [128, KT, HPC, 1], F32)
            nc.vector.tensor_copy(v_sb[:, :, :, 64:65], ones_col)

            # DRAM bounce buffers
            ot_ch = [dram.tile([FC, 512], BF16, name=f"ot{i}") for i in range(NT)]
            og_ch = [
                dram.tile([4 * FC, 512], BF16, name=f"og{i}") for i in range(NT)
            ]

            xts = []

            # ---- emitters ---------------------------------------------
            def emit_x_load(nt):
                xt_t = xtp.tile([128, ET, 512], F16, tag="xt")
                xts.append(xt_t)
                nc.sync.dma_start(out=xt_t[:], in_=xt_d[nt])

            def emit_qk(nt, pair, which):
                # which: 0 -> q, 1 -> k
                ps = eps_pool.tile([128, 512], F32, tag="e")
                off = which * 256 + pair * 128
                for kt in range(ET):
                    nc.tensor.matmul(
                        ps[:],
                        lhsT=wqkv_sb[:, kt, off : off + 128],
                        rhs=xts[nt][:, kt, :],
                        start=(kt == 0),
                        stop=(kt == ET - 1),
                    )
                dst = qt_sb if which == 0 else kt_sb
                nc.vector.tensor_copy(
                    dst[:, pair, nt * 512 : (nt + 1) * 512], ps[:]
                )

            def emit_v(nt, m):
                ps = eps_pool.tile([128, FC], F32, tag="e")
                for kt in range(ET):
                    nc.tensor.matmul(
                        ps[:],
                        lhsT=xts[nt][:, kt, m * 128 : (m + 1) * 128],
                        rhs=wqkv_sb[:, kt, 512:768],
                        start=(kt == 0),
                        stop=(kt == ET - 1),
                    )
                nc.vector.tensor_copy(
                    v_sb[:, nt * 4 + m, :, 0:64],
                    ps[:].rearrange("p (h d) -> p h d", h=HPC),
                )

            def attn_slot(qt, kt, pvt):
                # pvt: list of 4 per-head PV psum accumulators [65, 512]
                q_sl = slice(qt * 512, (qt + 1) * 512)
                exps = []
                for pair in range(2):
                    ep = eps_pool.tile([128, 1024], F32, tag="e")
                    for s in range(2):
                        d_sl = slice(s * 64, (s + 1) * 64)
                        nc.tensor.matmul(
                            ep[:, s * 512 : (s + 1) * 512],
                            lhsT=kt_sb[d_sl, pair, kt * 128 : (kt + 1) * 128],
                            rhs=qt_sb[d_sl, pair, q_sl],
                            start=True,
                            stop=True,
                        )
                    ex = expp.tile([128, 1024], BF16, tag="exp")
                    nc.scalar.activation(
                        ex[:], ep[:], mybir.ActivationFunctionType.Exp
                    )
                    exps.append(ex)
                for pair in range(2):
                    for s in range(2):
                        nc.tensor.matmul(
                            pvt[2 * pair + s][0:65, :],
                            lhsT=v_sb[:, kt, 2 * pair + s, :],
                            rhs=exps[pair][:, s * 512 : (s + 1) * 512],
                            start=(kt == 0),
                            stop=(kt == KT - 1),
                        )

            def emit_norm(qt, pvt):
                # normalize out^T[d, q] by 1/denominator[q] and store to the
                # AllGather input chunk.
                # 1) evacuate PV psum to SBUF immediately so the psum banks
                #    free up for the next qt's accumulators.  denominators go
                #    through the DVE (heads the DMA chain ASAP); the big pv
                #    copies run on the otherwise-idle ACT engine in parallel.
                den_sb = normp.tile([1, 2048], F32, tag="den_sb")
                pv_sb = normp.tile([64, 2048], F32, tag="pv_sb")
                for i in range(4):
                    c_sl = slice(i * 512, (i + 1) * 512)
                    nc.vector.tensor_copy(den_sb[:, c_sl], pvt[i][64:65, :])
                    nc.scalar.copy(pv_sb[:, c_sl], pvt[i][0:64, :])
                # 2) reciprocal on a [128, 16] scatter so the DVE is
                #    full-width (one DRAM bounce round-trip for all 4 heads)
                den_d = dram.tile([1, 2048], F32, tag="den", bufs=2)
                nc.sync.dma_start(out=den_d[:], in_=den_sb[:])
                den_sc = normp.tile([128, 16], F32, tag="den_sc")
                nc.sync.dma_start(
                    out=den_sc[:],
                    in_=bass.AP(
                        tensor=den_d.tensor,
                        offset=den_d.offset,
                        ap=[[16, 128], [1, 16]],
                    ),
                )
                rec_sc = normp.tile([128, 16], F32, tag="rec_sc")
                nc.vector.reciprocal(rec_sc[:], den_sc[:])
                rec_d = dram.tile([1, 2048], F32, tag="rec", bufs=2)
                nc.sync.dma_start(
                    out=bass.AP(
                        tensor=rec_d.tensor,
                        offset=rec_d.offset,
                        ap=[[16, 128], [1, 16]],
                    ),
                    in_=rec_sc[:],
                )
                rep = normp.tile([64, 2048], F32, tag="rep")
                nc.sync.dma_start(
                    out=rep[:],
                    in_=bass.AP(
                        tensor=rec_d.tensor,
                        offset=rec_d.offset,
                        ap=[[0, 64], [1, 2048]],
                    ),
                )
                # 3) normalize + store: one DMA per pair into the AG chunk
                o_sb = normp.tile([64, 2048], BF16, tag="o")
                nc.vector.tensor_mul(o_sb[:], pv_sb[:], rep[:])
                for pair in range(2):
                    nc.sync.dma_start(
                        out=ot_ch[qt][
                            2 * pair * 64 : 2 * (pair + 1) * 64, :
                        ].rearrange("(s dd) q -> dd s q", s=2),
                        in_=o_sb[:, pair * 1024 : (pair + 1) * 1024].rearrange(
                            "dd (s q) -> dd s q", s=2
                        ),
                    )

            def emit_ag(qt):
                nc.gpsimd.collective_compute(
                    "AllGather",
                    mybir.AluOpType.bypass,
                    replica_groups=GROUPS,
                    ins=[ot_ch[qt].opt()],
                    outs=[og_ch[qt].opt()],
                )

            proj_rhs = {}

            def emit_proj_rhs(ch):
                rhs_t = prhs_pool.tile([128, ET, 512], BF16, tag="prhs")
                proj_rhs[ch] = rhs_t
                # gpsimd queue: this DMA waits on the AllGather; keep that
                # wait off the sync queue so norm-chain DMAs never stall
                nc.gpsimd.dma_start(
                    out=rhs_t[:],
                    in_=og_ch[ch][:].rearrange("(k p) n -> p k n", p=128),
                )

            def emit_proj_mg(ch, mg):
                pps = eps_pool.tile([128, 512], F32, tag="e")
                for kt in range(ET):
                    nc.tensor.matmul(
                        pps[:],
                        lhsT=wproj_sb[:, kt, mg * 128 : (mg + 1) * 128],
                        rhs=proj_rhs[ch][:, kt, :],
                        start=(kt == 0),
                        stop=(kt == ET - 1),
                    )
                ob = outp.tile([128, 512], BF16, tag="ob")
                nc.vector.tensor_scalar(
                    ob[:],
                    pps[:],
                    bias_sb[:, mg : mg + 1],
                    0.0,
                    mybir.AluOpType.add,
                    mybir.AluOpType.max,
                )
                nc.sync.dma_start(
                    out=out_d[mg * 128 : (mg + 1) * 128, ch * 512 : (ch + 1) * 512],
                    in_=ob[:],
                )

            # ---- fused schedule ---------------------------------------
            # lead-in: per n-tile produce K,V (and Q for qt0), and run qt0's
            # attention k-tiles as soon as their K/V exist.
            pv_tiles = {}

            def alloc_pv(qt):
                pv_tiles[qt] = [
                    pvps_pool.tile([65, 512], F32, tag="pv", name=f"pv{qt}_{i}")
                    for i in range(4)
                ]

            alloc_pv(0)
            for nt in range(NT):
                emit_x_load(nt)
                if nt == 0:
                    # after x(nt0) so the first K matmuls start ASAP
                    nc.sync.dma_start(out=wqkv_sb[:], in_=wqkv_d[:])
                emit_qk(nt, 0, 1)  # k pair 0
                emit_qk(nt, 1, 1)  # k pair 1
                for m in range(4):
                    emit_v(nt, m)
                if nt == 0:
                    emit_qk(0, 0, 0)  # q pair 0 (qt 0)
                    emit_qk(0, 1, 0)  # q pair 1
                if nt == 1:
                    emit_qk(1, 0, 0)  # q for qt 1 (the qt loop emits qt+1)
                    emit_qk(1, 1, 0)
                for kt in range(4 * nt, 4 * nt + 4):
                    attn_slot(0, kt, pv_tiles[0])

            # weights for the projection tail load behind the lead-in
            nc.sync.dma_start(out=wproj_sb[:], in_=wproj_d[:])
            nc.sync.dma_start(
                out=bias_sb, in_=bias_d[:].rearrange("(g p) -> p g", p=128)
            )

            emit_norm(0, pv_tiles[0])
            emit_ag(0)

            # proj chunks run with a TWO-qt lag so their matmuls never reach
            # the PE queue head before their AllGather has completed (a proj
            # matmul waiting on a collective would head-of-line-block every
            # later PE instruction).
            for qt in range(1, NT):
                alloc_pv(qt)
                for kt in range(KT):
                    attn_slot(qt, kt, pv_tiles[qt])
                    if kt == 4 and qt < NT - 1:
                        emit_qk(qt + 1, 0, 0)  # q for next qt
                    if kt == 6 and qt < NT - 1:
                        emit_qk(qt + 1, 1, 0)
                    if qt >= 2:
                        # qt==2 runs proj(0); qt==3 runs proj(1) mid-loop.
                        # placement at kt>=8 guarantees the AllGather (done
                        # ~20us into the NEXT qt at worst) has completed, so
                        # the proj matmuls never head-of-line-block the PE.
                        if kt == 8:
                            emit_proj_rhs(qt - 2)
                        if kt == 10:
                            emit_proj_mg(qt - 2, 0)
                        if kt == 12:
                            emit_proj_mg(qt - 2, 1)
                    if qt == NT - 1 and kt == 14:
                        emit_proj_rhs(qt - 1)
                if qt < NT - 1:
                    emit_norm(qt, pv_tiles[qt])
                    emit_ag(qt)
                else:
                    # tail: norm(3)+AG(3) run on ACT/DVE/DMA/cc while the PE
                    # finishes proj(2); proj(3) follows once AG(3) lands.
                    emit_norm(qt, pv_tiles[qt])
                    emit_ag(qt)
                    emit_proj_mg(qt - 1, 0)
                    emit_proj_mg(qt - 1, 1)
            # final chunk: split the gathered-rhs DMA so the projection
            # matmuls start as soon as the first half lands
            ch = NT - 1
            rhs_t = prhs_pool.tile([128, ET, 512], BF16, tag="prhs")
            for half in range(2):
                e_sl = slice(half * 4, (half + 1) * 4)
                nc.gpsimd.dma_start(
                    out=rhs_t[:, e_sl, :],
                    in_=og_ch[ch][half * 512 : (half + 1) * 512, :].rearrange(
                        "(k p) n -> p k n", p=128
                    ),
                )
            pps = [
                eps_pool.tile([128, 512], F32, tag="e", name=f"ppst{i}")
                for i in range(2)
            ]
            for half in range(2):
                for kt in range(half * 4, half * 4 + 4):
                    for mg in range(2):
                        nc.tensor.matmul(
                            pps[mg][:],
                            lhsT=wproj_sb[:, kt, mg * 128 : (mg + 1) * 128],
                            rhs=rhs_t[:, kt, :],
                            start=(kt == 0),
                            stop=(kt == ET - 1),
                        )
            for mg in range(2):
                ob = outp.tile([128, 512], BF16, tag="ob")
                nc.vector.tensor_scalar(
                    ob[:],
                    pps[mg][:],
                    bias_sb[:, mg : mg + 1],
                    0.0,
                    mybir.AluOpType.add,
                    mybir.AluOpType.max,
                )
                nc.sync.dma_start(
                    out=out_d[
                        mg * 128 : (mg + 1) * 128, ch * 512 : (ch + 1) * 512
                    ],
                    in_=ob[:],
                )

    nc.compile()
    return nc


def _get_nc():
    global _CACHED_NC
    if _CACHED_NC is None:
        _CACHED_NC = _build()
    return _CACHED_NC


def _prep_inputs(x, w_qkv, w_proj, b_proj):
    """Shard + relayout the full inputs for the 8 cores."""
    x = np.asarray(x, dtype=np.float32)
    w_qkv = np.asarray(w_qkv, dtype=np.float32)
    w_proj = np.asarray(w_proj, dtype=np.float32)
    b_proj = np.asarray(b_proj, dtype=np.float32)

    # x^T per batch re-laid out as [NT, 128, ET, 512] so each n-tile loads
    # as one contiguous DMA burst
    xts = [
        np.ascontiguousarray(
            x[b].T.reshape(ET, 128, NT, 512).transpose(2, 1, 0, 3)
        ).astype(np.float16)
        for b in range(B)
    ]
    # w_qkv rows are (h, d, qkv)-interleaved with qkv innermost
    wr = w_qkv.reshape(H, D, 3, E)
    # fold the post-softmax 1/sqrt(E) scaling into w_proj
    wp = w_proj / np.sqrt(E).astype(np.float32)

    wqkv_shards, wproj_shards, bias_shards = [], [], []
    for r in range(4):
        heads = range(4 * r, 4 * r + 4)
        qrows = np.concatenate([wr[h, :, 0, :] for h in heads], 0)  # [256, E]
        krows = np.concatenate([wr[h, :, 1, :] for h in heads], 0)
        vrows = np.concatenate([wr[h, :, 2, :] for h in heads], 0)
        w_core = np.concatenate([qrows, krows, vrows], 0)  # [768, E]
        wqkv_shards.append(
            np.ascontiguousarray(
                w_core.T.reshape(ET, 128, QKV_F).transpose(1, 0, 2)
            ).astype(np.float16)
        )
        wproj_shards.append(
            np.ascontiguousarray(
                wp[r * FC : (r + 1) * FC, :].T.reshape(ET, 128, FC).transpose(
                    1, 0, 2
                )
            ).astype(ml_dtypes.bfloat16)
        )
        bias_shards.append(np.ascontiguousarray(b_proj[r * FC : (r + 1) * FC]))

    in_maps = []
    for c in range(NCORES):
        b, r = c // 4, c % 4
        in_maps.append(
            {
                "xt": xts[b],
                "wqkvt": wqkv_shards[r],
                "wprojt": wproj_shards[r],
                "bias": bias_shards[r],
            }
        )
    return in_maps


def kernel(x, w_qkv, w_proj, b_proj):
    global LAST_EXEC_NS, LAST_RESULTS
    nc = _get_nc()
    in_maps = _prep_inputs(x, w_qkv, w_proj, b_proj)
    trace = bool(int(os.environ.get("BASS_KERNEL_TRACE", "0")))
    res = run_bass_kernel_spmd(
        nc, in_maps, list(range(NCORES)), trace=trace
    )
    LAST_EXEC_NS = res.exec_time_ns
    LAST_RESULTS = res

    out = np.empty((B, N, E), dtype=np.float32)
    for g in range(B):
        pt = np.concatenate(
            [
                res.results[4 * g + r]["out"].astype(np.float32)
                for r in range(4)
            ],
            axis=0,
        )  # [1024 f, 2048 n]
        out[g] = pt.T
    return out
